# revision 1
# baseline (speedup 1.0000x reference)
"""Trainium2 Bass kernel for BayesianChangePointDetector (segment_reduce).

Contract: kernel(**inputs) takes FULL inputs (x:[128,8192,32] f32, plus 3
scalar prior params) and returns the FULL [128] f32 output. The batch dim is
sharded across 8 NeuronCores (16 rows each, pure data parallel, no
collectives); the host concatenates the 8 per-core [16] outputs.

Fast path (pm == 0, the shipped configuration): x is staged to the device in
fp16 (host-side cast; quantization error ~7e-4 against the 2e-2 gate), which
halves the HBM stream to ~23.3us/core. The N=32 feature reduce runs as an
fp16 pairwise fold tree on DVE (2x packed-mode), prefix sums use
tensor_tensor_scan (fp32 state) with the cross-partition/row carry fixed via
a triangular-ones matmul on PE, and the Bayes-factor assembly is fused into
per-row scalar_tensor_tensor/tensor_scalar ops spread across DVE/Pool/Act so
every engine stays under the per-chunk DMA time. bf is shifted by the
expected whole-window log-marginal (slot algebra) so exp needs no per-row max
bias; the data-dependent residual enters only the final sigmoid.

A general-pm fallback keeps the original f32 kernel.
"""

import sys

if "/opt/trn_rl_repo" not in sys.path:
    sys.path.insert(0, "/opt/trn_rl_repo")

import math
from contextlib import ExitStack

import numpy as np

import concourse.bass as bass
import concourse.tile as tile
from concourse import mybir

F32 = mybir.dt.float32
F16 = mybir.dt.float16
AF = mybir.ActivationFunctionType
ALU = mybir.AluOpType
AX = mybir.AxisListType

B, T, N = 128, 8192, 32
NCORES = 8
BL = B // NCORES  # 16 batch rows per core
P = 128           # partitions = t-blocks
U = T // P        # 64 t's per partition
NS = 32
NEG = -1.0e30

# fast-path batch chunking (rows per chunk); last chunk small for short tail
CHUNKS_FAST = [4, 4, 4, 4]

# near-end threshold: mask P_split > 6553  <=>  g >= 6553 (g = 64p+u)
NE_P0 = 6553 // U          # 102
NE_U0 = 6553 - NE_P0 * U   # 25

# analysis hook: called with a label at each emission-phase boundary
STAGE_CB = None

# tunables (swept offline; the shipped values won the TimelineSim sweep)
CFG = {
    "aside": "perb",      # perb | chunk_pool | chunk_dve
    "p5": "pool",         # pool | dve
    "s3": "dve",          # pool | dve
    "order": "coarse",    # coarse | ladder | explicit list via "order_list"
    "prep_pos": 3,
    "c0parts": 4,
    "stt_pool_rows": 0,
    "order_list": [
        ('f1h', 0, 0),
        ('f1h', 3, 0),
        ('f1h', 1, 0),
        ('f1h', 1, 1),
        ('f1h', 2, 0),
        ('f1h', 0, 1),
        ('f1h', 3, 1),
        ('f1h', 3, 2),
        ('f1h', 3, 3),
        ('f2h', 1, 0),
        ('f1h', 0, 2),
        ('f2h', 1, 1),
        ('ff', 1),
        ('f1h', 0, 3),
        ('f1h', 2, 1),
        ('f2h', 2, 0),
        ('f2h', 3, 0),
        ('f2h', 0, 0),
        ('f2h', 0, 1),
        ('scanA', 1),
        ('scanB', 1),
        ('f2h', 2, 1),
        ('ff', 0),
        ('scanA', 0),
        ('ff', 2),
        ('f2h', 3, 1),
        ('ff', 3),
        ('scanA', 2),
        ('off', 1),
        ('scanB', 2),
        ('scanB', 0),
        ('stt', 1),
        ('pool', 1),
        ('off', 2),
        ('scanA', 3),
        ('scanB', 3),
        ('off', 0),
        ('stt', 2),
        ('s3', 1),
        ('exp', 1),
        ('off', 3),
        ('pool', 2),
        ('stt', 0),
        ('pool', 0),
        ('ttr', 1),
        ('stt', 3),
        ('s3', 2),
        ('pool', 3),
        ('exp', 2),
        ('s3', 0),
        ('ttr', 2),
        ('s3', 3),
        ('exp', 3),
        ('ttr', 3),
        ('bfw',),
        ('exp', 0),
        ('fin', 3),
        ('ttr', 0),
        ('fin', 2),
        ('fin', 1),
        ('fin', 0),
    ],
}


def _mark(nc, label):
    if STAGE_CB is not None:
        STAGE_CB(label, nc.next_id())


def build_body_fast(ctx, tc, x, params, utc, idc, out):
    """pm == 0 path. x is fp16 [BL, T, N]."""
    nc = tc.nc
    pers = ctx.enter_context(tc.tile_pool(name="pers", bufs=1))
    xp = ctx.enter_context(tc.tile_pool(name="xp", bufs=1))
    wk = ctx.enter_context(tc.tile_pool(name="wk", bufs=1))
    psp = ctx.enter_context(tc.tile_pool(name="psp", bufs=1, space="PSUM"))
    ps1 = ctx.enter_context(tc.tile_pool(name="ps1", bufs=1, space="PSUM"))

    # ---------- DMAs: chunk0 x quarters first (earliest fold start), then
    # params/ut/id, then the remaining chunks ----------
    ptile = pers.tile([P, 3], F32)

    chunks = []
    o = 0
    for c in CHUNKS_FAST:
        chunks.append((o, c))
        o += c
    assert o == BL

    ut_t = pers.tile([P, P], F32)
    id_t = pers.tile([P, P], F32)
    ones_t = pers.tile([P, P], F32)
    nc.gpsimd.memset(ones_t[:], 1.0)

    xts = []
    for ci, (bs, bc) in enumerate(chunks):
        xt = xp.tile([P, bc, U, N], F16, tag=f"xt_{ci}")
        src = x[bs : bs + bc].rearrange("b (p u) n -> p b u n", p=P)
        if ci == 0:
            nparts = CFG.get("c0parts", 4)
        elif ci == len(chunks) - 1:
            nparts = 4
        else:
            nparts = 2
        QU = U // nparts
        for q in range(nparts):
            nc.sync.dma_start(
                xt[:, :, q * QU : (q + 1) * QU, :],
                src[:, :, q * QU : (q + 1) * QU, :],
            )
        if ci == 0:
            nc.sync.dma_start(ptile[:], params[:])
            nc.sync.dma_start(ut_t[:], utc[:])
            nc.sync.dma_start(id_t[:], idc[:])
        xts.append(xt)

    # ---------- scalar slots (computed on all partitions) ----------
    sv = pers.tile([P, NS], F32)
    tmp = pers.tile([P, 8], F32)

    def s(i):
        return sv[:, i : i + 1]

    def tm(i):
        return tmp[:, i : i + 1]

    def sb(i, np_=P, p0=0):
        return sv[p0 : p0 + np_, i : i + 1]

    coef = {}

    def emit_prep():
        # slots: 1 inv_nv, 2 inv_pv, 3 -inv_nv, 4 zRb, 5 k, 7 -inv_nv/2048,
        # 8 k^2/2, 9 inv_nv/2048, 13 pvW, 14 L2pinv, 15 Lpv, 16 LpvW,
        # 17 8192*inv_nv, 18 inv_nv/8192, 19 bfWc, 20 pv, 21 nv, 24 s19-LWC=128*inv_nv
        nc.scalar.activation(tm(0), ptile[:, 1:2], AF.Exp)
        nc.vector.tensor_scalar_add(tm(0), tm(0), 1.0)
        nc.scalar.activation(s(20), tm(0), AF.Ln)
        nc.scalar.activation(tm(1), ptile[:, 2:3], AF.Exp)
        nc.vector.tensor_scalar_add(tm(1), tm(1), 1.0)
        nc.scalar.activation(s(21), tm(1), AF.Ln)
        nc.vector.reciprocal(s(1), s(21))
        nc.vector.reciprocal(s(2), s(20))
        nc.vector.tensor_scalar_mul(s(3), s(1), -1.0)
        nc.vector.tensor_scalar(s(4), s(1), 8191.0, s(2), ALU.mult, ALU.add)
        nc.vector.tensor_scalar_mul(s(5), s(1), 1.0 / 32.0)
        nc.vector.tensor_scalar_mul(s(7), s(1), -0.5 / 1024.0)
        nc.vector.tensor_scalar_mul(s(9), s(1), 0.5 / 1024.0)
        nc.vector.tensor_mul(tm(0), s(5), s(5))
        nc.vector.tensor_scalar_mul(s(8), tm(0), 0.5)
        nc.scalar.activation(s(14), s(21), AF.Ln, scale=2.0 * math.pi)
        nc.scalar.activation(s(15), s(20), AF.Ln)
        nc.vector.tensor_scalar_mul(s(17), s(1), 8192.0)
        nc.vector.tensor_scalar(tm(2), s(1), 8192.0, s(2), ALU.mult, ALU.add)
        nc.vector.reciprocal(s(13), tm(2))
        nc.scalar.activation(s(16), s(13), AF.Ln)
        nc.vector.tensor_scalar_mul(s(18), s(1), 1.0 / 8192.0)
        # s19 = -4096*L2pinv + 0.5*(LpvW - Lpv);  s24 = s19 - LWC = 128*inv_nv
        nc.vector.tensor_sub(tm(5), s(16), s(15))
        nc.vector.tensor_scalar_mul(tm(5), tm(5), 0.5)
        nc.vector.tensor_scalar(tm(6), s(14), -4096.0, tm(5), ALU.mult, ALU.add)
        nc.vector.tensor_copy(s(19), tm(6))
        nc.vector.tensor_scalar_mul(s(24), s(1), 128.0)

        _mark(nc, "prep_coef")
        # ---------- per-candidate coefficient vectors [P, U] ----------
        gti = pers.tile([P, U], mybir.dt.int32)
        nc.gpsimd.iota(gti[:], [[1, U]], base=0, channel_multiplier=U)
        gt = pers.tile([P, U], F32)
        nc.vector.tensor_copy(gt[:], gti[:])

        nf = pers.tile([P, U], F32)
        nc.gpsimd.tensor_scalar_add(nf[:], gt[:], 1.0)
        zL = pers.tile([P, U], F32)
        nc.vector.tensor_scalar(zL[:], nf[:], sb(1), sb(2), ALU.mult, ALU.add)
        pvnL = pers.tile([P, U], F32)
        nc.vector.reciprocal(pvnL[:], zL[:])
        zR = pers.tile([P, U], F32)
        nc.vector.tensor_scalar(zR[:], gt[:], sb(3), sb(4), ALU.mult, ALU.add)
        pvnR = pers.tile([P, U], F32)
        nc.vector.reciprocal(pvnR[:], zR[:])
        lpvnL = pers.tile([P, U], F32)
        nc.scalar.activation(lpvnL[:], pvnL[:], AF.Ln)
        lpvnR = pers.tile([P, U], F32)
        nc.scalar.activation(lpvnR[:], pvnR[:], AF.Ln)
        kc2 = pers.tile([P, U], F32)
        nc.gpsimd.tensor_add(kc2[:], lpvnL[:], lpvnR[:])

        nRf = pers.tile([P, U], F32)
        nc.gpsimd.tensor_scalar(nRf[:], gt[:], -1.0, 8191.0, ALU.mult, ALU.add)
        gc = pers.tile([P, U], F32)
        nc.gpsimd.tensor_scalar_max(gc[:], gt[:], 1.0)
        inv_n1 = pers.tile([P, U], F32)
        nc.vector.reciprocal(inv_n1[:], gc[:])
        nR1c = pers.tile([P, U], F32)
        nc.gpsimd.tensor_scalar(nR1c[:], gt[:], -1.0, 8190.0, ALU.mult, ALU.add)
        nc.gpsimd.tensor_scalar_max(nR1c[:], nR1c[:], 1.0)
        inv_nR1 = pers.tile([P, U], F32)
        nc.vector.reciprocal(inv_nR1[:], nR1c[:])
        inv_n = pers.tile([P, U], F32)
        nc.vector.reciprocal(inv_n[:], nf[:])
        nRc = pers.tile([P, U], F32)
        nc.gpsimd.tensor_scalar_max(nRc[:], nRf[:], 1.0)
        inv_nR = pers.tile([P, U], F32)
        nc.vector.reciprocal(inv_nR[:], nRc[:])

        n_n1 = pers.tile([P, U], F32)
        nc.gpsimd.tensor_mul(n_n1[:], nf[:], inv_n1[:])
        nR_nR1 = pers.tile([P, U], F32)
        nc.gpsimd.tensor_mul(nR_nR1[:], nRf[:], inv_nR1[:])
        i_nn1 = pers.tile([P, U], F32)
        nc.gpsimd.tensor_mul(i_nn1[:], inv_n[:], inv_n1[:])
        i_nRnR1 = pers.tile([P, U], F32)
        nc.gpsimd.tensor_mul(i_nRnR1[:], inv_nR[:], inv_nR1[:])

        CA2L = pers.tile([P, U], F32)
        q1 = pers.tile([P, U], F32)
        nc.scalar.activation(q1[:], pvnL[:], AF.Copy, scale=sb(8))
        nc.vector.tensor_scalar(CA2L[:], i_nn1[:], sb(9), None, ALU.mult)
        nc.gpsimd.tensor_add(CA2L[:], CA2L[:], q1[:])
        CA2R = pers.tile([P, U], F32)
        q1b = pers.tile([P, U], F32)
        nc.scalar.activation(q1b[:], pvnR[:], AF.Copy, scale=sb(8))
        nc.vector.tensor_scalar(CA2R[:], i_nRnR1[:], sb(9), None, ALU.mult)
        nc.gpsimd.tensor_add(CA2R[:], CA2R[:], q1b[:])
        CBL = pers.tile([P, U], F32)
        nc.scalar.activation(CBL[:], n_n1[:], AF.Copy, scale=sb(7))
        CBR = pers.tile([P, U], F32)
        nc.scalar.activation(CBR[:], nR_nR1[:], AF.Copy, scale=sb(7))
        CBD = pers.tile([P, U], F32)
        nc.vector.tensor_sub(CBD[:], CBL[:], CBR[:])

        # Cc = 0.5*kc2 + (-4096*L2pinv - Lpv) - LWC + mask, with
        # LWC = s19 - 128*inv_nv  =>  const = 0.5*kc2 - Lpv - 0.5*(LpvW-Lpv)
        #                                  + 128*inv_nv
        Cc = pers.tile([P, U], F32)
        ccs = pers.tile([P, 1], F32)
        # ccs = -Lpv - 0.5*(LpvW - Lpv) + 128*inv_nv = -0.5*Lpv - 0.5*LpvW + s24
        nc.vector.tensor_add(tm(3), s(15), s(16))
        nc.vector.tensor_scalar(ccs[:], tm(3), -0.5, s(24), ALU.mult, ALU.add)
        nc.vector.tensor_scalar_mul(Cc[:], kc2[:], 0.5)
        mlo = pers.tile([P, U], F32)
        nc.vector.tensor_scalar(mlo[:], gt[:], 14.5, NEG, ALU.is_lt, ALU.mult)
        mhi = pers.tile([P, U], F32)
        nc.vector.tensor_scalar(mhi[:], gt[:], 8174.5, NEG, ALU.is_ge, ALU.mult)
        nc.gpsimd.tensor_add(Cc[:], Cc[:], mlo[:])
        nc.gpsimd.tensor_add(Cc[:], Cc[:], mhi[:])
        expCc = pers.tile([P, U], F32)
        nc.scalar.activation(expCc[:], Cc[:], AF.Exp)

        # partition masks for the near-end sum: a1 = (p > NE_P0), a2 = (p == NE_P0)
        pidx_i = pers.tile([P, 1], mybir.dt.int32)
        nc.gpsimd.iota(pidx_i[:], [[1, 1]], base=0, channel_multiplier=1)
        pidx = pers.tile([P, 1], F32)
        nc.vector.tensor_copy(pidx[:], pidx_i[:])
        a1 = pers.tile([P, 1], F32)
        nc.vector.tensor_scalar(a1[:], pidx[:], NE_P0 + 0.5, None, ALU.is_ge)
        a2 = pers.tile([P, 1], F32)
        nc.vector.tensor_scalar(a2[:], pidx[:], NE_P0 - 0.5, None, ALU.is_ge)
        nc.vector.tensor_sub(a2[:], a2[:], a1[:])

        zeros = pers.tile([P, max(CHUNKS_FAST) * U], F32)
        nc.gpsimd.memset(zeros[:], 0.0)
        coef['ccs'] = ccs
        coef['CA2L'] = CA2L
        coef['CA2R'] = CA2R
        coef['CBL'] = CBL
        coef['CBR'] = CBR
        coef['CBD'] = CBD
        coef['Cc'] = Cc
        coef['expCc'] = expCc
        coef['a1'] = a1
        coef['a2'] = a2
        coef['zeros'] = zeros

    # bund: [0:16) maxbf | [16:32) Zp | [32:48) En | [48:64) At | [64:80) Btot
    bund = pers.tile([P, 80], F32)

    # ---------- per-chunk pipeline, software-pipelined at fine granularity:
    # emission order approximates per-op data-ready order so every in-order
    # engine queue stays busy ----------
    st = [dict() for _ in chunks]  # per-chunk tile state

    def u_f1h(ci, h):
        """fold1 for DMA piece h of chunk ci (chunk0: 4 quarters, else halves)."""
        bs, bc = chunks[ci]
        xt = xts[ci]
        if ci == 0:
            nparts = CFG.get("c0parts", 4)
        elif ci == len(chunks) - 1:
            nparts = 4
        else:
            nparts = 2
        QU = U // nparts
        if h == 0:
            st[ci]["f1"] = wk.tile([P, bc, U, 16], F16, tag=f"f1_{ci}", name=f"f1_{ci}")
        f1 = st[ci]["f1"]
        qs = slice(h * QU, (h + 1) * QU)
        nc.vector.tensor_tensor(
            f1[:, :, qs, :], xt[:, :, qs, 0:16], xt[:, :, qs, 16:32], ALU.add
        )

    def u_f2h(ci, h):
        bs, bc = chunks[ci]
        f1 = st[ci]["f1"]
        HU = U // 2
        if h == 0:
            st[ci]["f2"] = wk.tile(
                [P, bc, U, 8], F16, tag=f"f2_{ci}", name=f"f2_{ci}"
            )
        f2 = st[ci]["f2"]
        hs = slice(h * HU, (h + 1) * HU)
        nc.vector.tensor_tensor(
            f2[:, :, hs, :], f1[:, :, hs, 0:8], f1[:, :, hs, 8:16], ALU.add
        )

    def u_ff(ci):
        bs, bc = chunks[ci]
        f2 = st[ci]["f2"]
        f3 = wk.tile([P, bc, U, 4], F16, tag=f"f3_{ci}")
        nc.vector.tensor_tensor(f3[:], f2[:, :, :, 0:4], f2[:, :, :, 4:8], ALU.add)
        f4 = wk.tile([P, bc, U, 2], F16, tag=f"f4_{ci}")
        nc.vector.tensor_tensor(f4[:], f3[:, :, :, 0:2], f3[:, :, :, 2:4], ALU.add)
        sr = wk.tile([P, bc, U], F16, tag=f"sr_{ci}")
        nc.gpsimd.tensor_tensor(sr[:], f4[:, :, :, 0], f4[:, :, :, 1], ALU.add)
        sq = wk.tile([P, bc, U], F32, tag=f"sq_{ci}")
        nc.scalar.activation(sq[:], sr[:], AF.Square)
        st[ci].update(f4=f4, sq=sq)

    def u_scanA(ci):
        bs, bc = chunks[ci]
        f4 = st[ci]["f4"]
        A = wk.tile([P, bc, U], F32, tag=f"A_{ci}")
        nc.vector.tensor_tensor_scan(
            A[:].rearrange("p b u -> p (b u)"),
            f4[:, :, :, 0].rearrange("p b u -> p (b u)"),
            f4[:, :, :, 1].rearrange("p b u -> p (b u)"),
            0.0,
            ALU.add,
            ALU.add,
        )
        st[ci]["A"] = A

    def u_scanB(ci):
        bs, bc = chunks[ci]
        sq = st[ci]["sq"]
        A = st[ci]["A"]
        V = bc * U
        Bt = wk.tile([P, bc, U], F32, tag=f"Bt_{ci}")
        nc.vector.tensor_tensor_scan(
            Bt[:].rearrange("p b u -> p (b u)"),
            sq[:].rearrange("p b u -> p (b u)"),
            coef["zeros"][:, 0:V],
            0.0,
            ALU.add,
            ALU.add,
        )
        rv = wk.tile([P, 2 * bc], F32, tag=f"rv_{ci}")
        if bc > 1:
            nc.gpsimd.memset(rv[:, 0:1], 0.0)
            nc.gpsimd.memset(rv[:, bc : bc + 1], 0.0)
            nc.vector.tensor_copy(rv[:, 1:bc], A[:, 0 : bc - 1, U - 1])
            nc.vector.tensor_copy(rv[:, bc + 1 : 2 * bc], Bt[:, 0 : bc - 1, U - 1])
        else:
            nc.gpsimd.memset(rv[:], 0.0)
        ct = wk.tile([P, 2 * bc], F32, tag=f"ct_{ci}")
        nc.vector.tensor_sub(ct[:, 0:bc], A[:, :, U - 1], rv[:, 0:bc])
        nc.vector.tensor_sub(ct[:, bc : 2 * bc], Bt[:, :, U - 1], rv[:, bc : 2 * bc])
        mm = psp.tile([P, 2, 2 * bc], F32, tag=f"mm_{ci}")
        nc.tensor.matmul(mm[:, 0, :], ut_t[:], ct[:])
        nc.tensor.matmul(mm[:, 1, :], ones_t[:], ct[:])
        nc.scalar.copy(
            bund[:, 48:80].rearrange("p (s c) -> p s c", s=2)[:, :, bs : bs + bc],
            mm[:, 1, :].rearrange("p (s c) -> p s c", s=2),
        )
        st[ci].update(Bt=Bt, rv=rv, mm=mm)

    def u_off(ci):
        bs, bc = chunks[ci]
        A = st[ci]["A"]
        rv = st[ci]["rv"]
        mm = st[ci]["mm"]
        offA = wk.tile([P, bc], F32, tag=f"offA_{ci}")
        nc.vector.tensor_sub(offA[:], mm[:, 0, 0:bc], rv[:, 0:bc])
        noffB = wk.tile([P, bc], F32, tag=f"noffB_{ci}")
        nc.vector.tensor_sub(noffB[:], rv[:, bc : 2 * bc], mm[:, 0, bc : 2 * bc])
        Sp = wk.tile([P, bc], F32, tag=f"Sp_{ci}")
        nc.vector.tensor_sub(Sp[:], mm[:, 1, 0:bc], offA[:])
        def pb(t):
            return t[:].unsqueeze(2).broadcast_to([P, bc, U])

        AR = wk.tile([P, bc, U], F32, tag=f"AR_{ci}")
        A2 = wk.tile([P, bc, U], F32, tag=f"A2_{ci}")
        if CFG["aside"] == "perb":
            for b in range(bc):
                nc.gpsimd.tensor_scalar(
                    AR[:, b], A[:, b], -1.0, Sp[:, b : b + 1], ALU.mult, ALU.add
                )
                nc.scalar.activation(
                    A2[:, b], A[:, b], AF.Square, bias=offA[:, b : b + 1]
                )
        else:
            eng = nc.gpsimd if CFG["aside"] == "chunk_pool" else nc.vector
            At_t = wk.tile([P, bc, U], F32, tag=f"Att_{ci}", name=f"Att_{ci}")
            eng.tensor_add(At_t[:], A[:], pb(offA))
            eng.tensor_sub(AR[:], pb(Sp), A[:])
            nc.scalar.activation(A2[:], At_t[:], AF.Square)
        AR2 = wk.tile([P, bc, U], F32, tag=f"AR2_{ci}")
        nc.scalar.activation(AR2[:], AR[:], AF.Square)
        st[ci].update(noffB=noffB, A2=A2, AR2=AR2)

    def u_stt(ci):
        bs, bc = chunks[ci]
        Bt = st[ci]["Bt"]
        noffB = st[ci]["noffB"]
        mm = st[ci]["mm"]
        s2t = wk.tile([P, bc, U], F32, tag=f"s2t_{ci}")
        mt = wk.tile([P, bc, U], F32, tag=f"mt_{ci}")
        kp = CFG.get("stt_pool_rows", 0)
        for b in range(bc):
            if b < kp:
                nc.gpsimd.tensor_scalar(
                    mt[:, b], Bt[:, b], 1.0, noffB[:, b : b + 1], ALU.mult,
                    ALU.subtract,
                )
                nc.gpsimd.tensor_mul(mt[:, b], mt[:, b], coef["CBD"][:])
            else:
                nc.vector.scalar_tensor_tensor(
                    mt[:, b], Bt[:, b], noffB[:, b : b + 1], coef["CBD"][:],
                    ALU.subtract, ALU.mult,
                )
            nc.vector.scalar_tensor_tensor(
                s2t[:, b], coef["CBR"][:], mm[:, 1, bc + b : bc + b + 1], mt[:, b],
                ALU.mult, ALU.add,
            )
        st[ci].update(s2t=s2t, mt=mt)

    def u_pool(ci):
        # p2 on Pool; p5 + partial sum on DVE (fills DVE's rendezvous slack)
        bs, bc = chunks[ci]
        A2 = st[ci]["A2"]
        AR2 = st[ci]["AR2"]

        def cb(t):
            return t[:].unsqueeze(1).broadcast_to([P, bc, U])

        p2 = wk.tile([P, bc, U], F32, tag=f"p2_{ci}")
        nc.gpsimd.tensor_mul(p2[:], A2[:], cb(coef["CA2L"]))
        p5 = wk.tile([P, bc, U], F32, tag=f"p5_{ci}")
        e5 = nc.vector if CFG["p5"] == "dve" else nc.gpsimd
        e5.tensor_mul(p5[:], AR2[:], cb(coef["CA2R"]))
        e5.tensor_add(st[ci]["s2t"][:], st[ci]["s2t"][:], p5[:])
        st[ci].update(p2=p2, p5=p5)

    def u_s3(ci):
        bs, bc = chunks[ci]
        e3 = nc.vector if CFG["s3"] == "dve" else nc.gpsimd
        e3.tensor_add(st[ci]["s2t"][:], st[ci]["s2t"][:], st[ci]["p2"][:])

    def u_exp(ci):
        bs, bc = chunks[ci]
        es = st[ci]["mt"]  # reuse
        nc.scalar.activation(es[:], st[ci]["s2t"][:], AF.Exp, bias=coef["ccs"][:])
        st[ci]["es"] = es

    def u_ttr(ci):
        # e = exp(s3)*exp(Cc) (mask exact-zero) with per-row Zp accumulated
        bs, bc = chunks[ci]
        es = st[ci]["es"]
        e = st[ci]["p5"]  # reuse
        for b in range(bc):
            nc.vector.scalar_tensor_tensor(
                e[:, b], es[:, b], 1.0, coef["expCc"][:], ALU.mult, ALU.mult,
                accum_out=bund[:, 16 + bs + b : 17 + bs + b],
            )
        st[ci]["e"] = e

    def u_fin(ci):
        bs, bc = chunks[ci]
        e = st[ci]["e"]
        nc.vector.tensor_reduce(bund[:, bs : bs + bc], e[:], AX.X, ALU.max)
        ssuf = wk.tile([P, bc], F32, tag=f"ssuf_{ci}")
        nc.vector.tensor_reduce(ssuf[:], e[:, :, NE_U0:U], AX.X, ALU.add)
        t2 = wk.tile([P, bc], F32, tag=f"t2_{ci}")
        nc.vector.tensor_scalar(t2[:], ssuf[:], coef["a2"][:], None, ALU.mult)
        nc.vector.scalar_tensor_tensor(
            bund[:, 32 + bs : 32 + bs + bc],
            bund[:, 16 + bs : 16 + bs + bc], coef["a1"][:], t2[:],
            ALU.mult, ALU.add,
        )

    Tall = ps1.tile([BL, 5, P], F32)
    rebfW = pers.tile([BL, 1], F32)
    erb = pers.tile([BL, 1], F32)

    def u_bfw():
        """whole-window log-marginal residual; only needs bund At/Bt columns
        (complete after the last chunk's u_scanB) — runs off the tail."""
        nc.tensor.transpose(Tall[:, 3, :], bund[:, 48 : 48 + BL], id_t[:])
        nc.tensor.transpose(Tall[:, 4, :], bund[:, 64 : 64 + BL], id_t[:])
        At16 = Tall[:, 3, 0:1]
        Bt16 = Tall[:, 4, 0:1]
        t1b = pers.tile([BL, 1], F32)
        nc.scalar.activation(t1b[:], At16, AF.Square, scale=1.0 / 32.0)
        v2 = pers.tile([BL, 1], F32)
        nc.vector.tensor_scalar_mul(v2[:], t1b[:], 1.0 / 8192.0)
        vW = pers.tile([BL, 1], F32)
        nc.vector.scalar_tensor_tensor(
            vW[:], Bt16, 1.0 / 1024.0, v2[:], ALU.mult, ALU.subtract
        )
        nc.vector.tensor_scalar(
            vW[:], vW[:], 1.0 / 8191.0, 1.0e-8, ALU.mult, ALU.max
        )
        term1 = pers.tile([BL, 1], F32)
        nc.vector.tensor_scalar(term1[:], vW[:], sb(17, BL), None, ALU.mult)
        term2 = pers.tile([BL, 1], F32)
        nc.vector.tensor_scalar(term2[:], t1b[:], sb(18, BL), None, ALU.mult)
        uu = pers.tile([BL, 1], F32)
        nc.vector.tensor_scalar(uu[:], At16, sb(5, BL), None, ALU.mult)
        u2 = pers.tile([BL, 1], F32)
        nc.vector.tensor_mul(u2[:], uu[:], uu[:])
        term3 = pers.tile([BL, 1], F32)
        nc.vector.tensor_scalar(term3[:], u2[:], sb(13, BL), None, ALU.mult)
        tsum = pers.tile([BL, 1], F32)
        nc.vector.tensor_add(tsum[:], term1[:], term2[:])
        nc.vector.tensor_sub(tsum[:], tsum[:], term3[:])
        nc.vector.tensor_scalar(
            rebfW[:], tsum[:], -0.5, sb(24, BL), ALU.mult, ALU.add
        )
        nc.scalar.activation(erb[:], rebfW[:], AF.Exp, scale=-1.0)

    units = {
        "f1h": u_f1h, "f2h": u_f2h, "ff": u_ff, "scanA": u_scanA,
        "scanB": u_scanB, "off": u_off, "stt": u_stt, "pool": u_pool,
        "s3": u_s3, "exp": u_exp, "ttr": u_ttr, "fin": u_fin, "bfw": u_bfw,
    }
    if CFG.get("order_list"):
        order = [tuple(t) for t in CFG["order_list"]]
    elif CFG["order"] == "ladder":
        order = [
            ("f1h", 0, 0), ("f1h", 0, 1), ("f2h", 0, 0),
            ("f1h", 0, 2), ("f1h", 0, 3), ("f2h", 0, 1),
            ("ff", 0), ("scanA", 0), ("scanB", 0),
            ("f1h", 1, 0), ("f2h", 1, 0),
            ("off", 0), ("stt", 0), ("pool", 0),
            ("f1h", 1, 1), ("f2h", 1, 1), ("ff", 1),
            ("s3", 0), ("exp", 0), ("scanA", 1), ("ttr", 0),
            ("scanB", 1), ("fin", 0),
            ("off", 1), ("stt", 1), ("pool", 1),
            ("f1h", 2, 0), ("f2h", 2, 0),
            ("s3", 1), ("exp", 1), ("ttr", 1), ("fin", 1),
            ("f1h", 2, 1), ("f2h", 2, 1), ("ff", 2),
            ("scanA", 2), ("scanB", 2),
            ("f1h", 3, 0), ("f1h", 3, 1), ("f2h", 3, 0),
            ("off", 2), ("stt", 2), ("pool", 2),
            ("f1h", 3, 2), ("f1h", 3, 3), ("f2h", 3, 1), ("ff", 3),
            ("scanA", 3), ("scanB", 3),
            ("s3", 2), ("exp", 2), ("ttr", 2), ("fin", 2),
            ("off", 3), ("stt", 3),
            ("bfw",),
            ("pool", 3), ("s3", 3), ("exp", 3), ("ttr", 3), ("fin", 3),
        ]
    else:
        NCH = len(chunks)
        order = []
        for ci in range(NCH):
            np_ = 4 if ci in (0, NCH - 1) else 2
            order += [("f1h", ci, h) for h in range(np_)]
            order += [("f2h", ci, 0), ("f2h", ci, 1), ("ff", ci)]
        # coarse wavefront over remaining stages
        STAGES = [["scanA", "scanB"], ["off", "stt", "pool"],
                  ["s3", "exp", "ttr"], ["fin"]]
        # interleave: folds already emitted above; now stage waves
        order = []
        for ci in range(NCH):
            np_ = 4 if ci in (0, NCH - 1) else 2
            fold_units = [("f1h", ci, h) for h in range(np_)] + [
                ("f2h", ci, 0), ("f2h", ci, 1), ("ff", ci)]
            order.append(("group", ci, fold_units))
        flat = []
        groups = {ci: g for (_, ci, g) in order}
        waves = []
        for w in range(len(chunks) + len(STAGES)):
            for ci in range(NCH):
                k = w - ci
                if k == 0:
                    waves.append(("g", ci))
                elif 1 <= k <= len(STAGES):
                    waves.append(("s", k - 1, ci))
        order = []
        for item in waves:
            if item[0] == "g":
                order += groups[item[1]]
            else:
                _, k, ci = item
                for uname in STAGES[k]:
                    order.append((uname, ci))
        order.insert(len(order) - 6, ("bfw",))
    prep_pos = CFG.get("prep_pos", 0)
    for idx, item in enumerate(order):
        if idx == prep_pos:
            emit_prep()
        _mark(nc, "_".join(str(z) for z in item))
        units[item[0]](*item[1:])
    if prep_pos >= len(order):
        emit_prep()
    _mark(nc, "finale")

    # ---------- finale tail (bfW already computed via u_bfw) ----------
    nc.tensor.transpose(Tall[:, 0, :], bund[:, 0:BL], id_t[:])
    nc.tensor.transpose(Tall[:, 1, :], bund[:, 16 : 16 + BL], id_t[:])
    nc.tensor.transpose(Tall[:, 2, :], bund[:, 32 : 32 + BL], id_t[:])
    M16 = pers.tile([BL, 1], F32)
    nc.vector.tensor_reduce(M16[:], Tall[:, 0, :], AX.X, ALU.max)
    Z16 = pers.tile([BL, 1], F32)
    nc.vector.tensor_reduce(Z16[:], Tall[:, 1, :], AX.X, ALU.add)
    E16 = pers.tile([BL, 1], F32)
    nc.vector.tensor_reduce(E16[:], Tall[:, 2, :], AX.X, ALU.add)

    # conf = sigmoid(ln(Me) - rebfW) = z/(1+z) with z = Me*exp(-rebfW)
    ze = pers.tile([BL, 1], F32)
    nc.vector.tensor_mul(ze[:], M16[:], erb[:])
    den = pers.tile([BL, 1], F32)
    nc.vector.tensor_scalar_add(den[:], ze[:], 1.0)
    invd = pers.tile([BL, 1], F32)
    nc.vector.reciprocal(invd[:], den[:])
    invZ = pers.tile([BL, 1], F32)
    nc.vector.reciprocal(invZ[:], Z16[:])
    ratio = pers.tile([BL, 1], F32)
    nc.vector.tensor_mul(ratio[:], E16[:], invZ[:])
    nc.vector.tensor_mul(ratio[:], ratio[:], ze[:])
    outv = pers.tile([BL, 1], F32)
    nc.vector.tensor_mul(outv[:], ratio[:], invd[:])
    nc.sync.dma_start(out[:], outv[:])


# ======================================================================
# general-pm fallback: the original f32 kernel (unchanged numerics)
# ======================================================================

BC = 4
NCHUNK = BL // BC
UF_SCHED = [16, 32, 32, 32]
CHUNK_SIZES = [4, 4, 4, 4]
XP_BUFS = 2
WK_BUFS = 3
LO_INV_U = 15
HI_INV_U = 8174 - 127 * U + 1


def build_body(ctx, tc, x, params, utc, idc, out, pm_zero=False):
    nc = tc.nc
    pers = ctx.enter_context(tc.tile_pool(name="pers", bufs=1))
    xp = ctx.enter_context(tc.tile_pool(name="xp", bufs=XP_BUFS))
    wk = ctx.enter_context(tc.tile_pool(name="wk", bufs=WK_BUFS))
    psp = ctx.enter_context(tc.tile_pool(name="psp", bufs=2, space="PSUM"))
    ps1 = ctx.enter_context(tc.tile_pool(name="ps1", bufs=1, space="PSUM"))

    ut_t = pers.tile([P, P], F32)
    ones_t = pers.tile([P, P], F32)
    id_t = pers.tile([P, P], F32)
    gt = pers.tile([P, U], F32)
    ptile = pers.tile([P, 3], F32)
    nc.sync.dma_start(ptile[:], params[:])
    nc.gpsimd.memset(ones_t[:], 1.0)
    gti = pers.tile([P, U], mybir.dt.int32)
    nc.gpsimd.iota(gti[:], [[1, U]], base=0, channel_multiplier=U)
    nc.vector.tensor_copy(gt[:], gti[:])

    chunks = []
    o = 0
    for c in CHUNK_SIZES:
        chunks.append((o, c))
        o += c
    assert o == BL
    xts = []
    for ci, (bs, bc) in enumerate(chunks):
        xt = xp.tile([P, bc, U, N], F32, tag="xt")
        src = x[bs : bs + bc].rearrange("b (p u) n -> p b u n", p=P)
        if ci == 0:
            QU = U // 4
            for q in range(4):
                nc.sync.dma_start(
                    xt[:, :, q * QU : (q + 1) * QU, :],
                    src[:, :, q * QU : (q + 1) * QU, :],
                )
        else:
            HU = U // 2
            nc.sync.dma_start(xt[:, :, 0:HU, :], src[:, :, 0:HU, :])
            nc.sync.dma_start(xt[:, :, HU:U, :], src[:, :, HU:U, :])
        if ci == 0:
            nc.sync.dma_start(ptile[:], params[:])
            nc.sync.dma_start(ut_t[:], utc[:])
            nc.sync.dma_start(id_t[:], idc[:])
        xts.append(xt)

    sv = pers.tile([P, NS], F32)
    tmp = pers.tile([P, 8], F32)

    def s(i):
        return sv[:, i : i + 1]

    def tm(i):
        return tmp[:, i : i + 1]

    nc.scalar.activation(tm(0), ptile[:, 1:2], AF.Exp)
    nc.vector.tensor_scalar_add(tm(0), tm(0), 1.0)
    nc.scalar.activation(s(20), tm(0), AF.Ln)
    nc.scalar.activation(tm(1), ptile[:, 2:3], AF.Exp)
    nc.vector.tensor_scalar_add(tm(1), tm(1), 1.0)
    nc.scalar.activation(s(21), tm(1), AF.Ln)
    nc.vector.tensor_copy(s(0), ptile[:, 0:1])
    nc.vector.reciprocal(s(1), s(21))
    nc.vector.reciprocal(s(2), s(20))
    nc.vector.tensor_scalar_mul(s(3), s(1), -1.0)
    nc.vector.tensor_scalar(s(4), s(1), 8191.0, s(2), ALU.mult, ALU.add)
    nc.vector.tensor_scalar_mul(s(5), s(1), 1.0 / 32.0)
    nc.vector.tensor_mul(s(6), s(0), s(2))
    nc.vector.tensor_scalar_mul(s(7), s(1), -0.5 / 1024.0)
    nc.vector.tensor_scalar_mul(s(9), s(1), 0.5 / 1024.0)
    nc.vector.tensor_mul(tm(0), s(5), s(5))
    nc.vector.tensor_scalar_mul(s(8), tm(0), 0.5)
    nc.vector.tensor_mul(s(10), s(6), s(5))
    nc.vector.tensor_mul(tm(1), s(6), s(6))
    nc.vector.tensor_scalar_mul(s(11), tm(1), 0.5)
    nc.scalar.activation(s(14), s(21), AF.Ln, scale=2.0 * math.pi)
    nc.scalar.activation(s(15), s(20), AF.Ln)
    nc.vector.tensor_scalar_mul(s(17), s(1), 8192.0)
    nc.vector.tensor_scalar(tm(2), s(1), 8192.0, s(2), ALU.mult, ALU.add)
    nc.vector.reciprocal(s(13), tm(2))
    nc.scalar.activation(s(16), s(13), AF.Ln)
    nc.vector.tensor_scalar_mul(s(18), s(1), 1.0 / 8192.0)
    nc.vector.tensor_mul(tm(3), s(0), s(0))
    nc.vector.tensor_mul(s(22), tm(3), s(2))
    nc.vector.tensor_scalar_mul(s(23), s(14), -4096.0)
    nc.vector.tensor_sub(tm(4), s(23), s(15))
    nc.vector.tensor_sub(s(12), tm(4), s(22))
    nc.vector.tensor_sub(tm(5), s(16), s(15))
    nc.vector.tensor_scalar_mul(tm(5), tm(5), 0.5)
    nc.vector.tensor_add(tm(6), s(23), tm(5))
    nc.vector.tensor_scalar_mul(tm(7), s(22), -0.5)
    nc.vector.tensor_add(s(19), tm(6), tm(7))

    def sb(i, np_=P, p0=0):
        return sv[p0 : p0 + np_, i : i + 1]

    nf = pers.tile([P, U], F32)
    nc.vector.tensor_scalar_add(nf[:], gt[:], 1.0)
    zL = pers.tile([P, U], F32)
    nc.vector.tensor_scalar(zL[:], nf[:], sb(1), sb(2), ALU.mult, ALU.add)
    pvnL = pers.tile([P, U], F32)
    nc.vector.reciprocal(pvnL[:], zL[:])
    zR = pers.tile([P, U], F32)
    nc.vector.tensor_scalar(zR[:], gt[:], sb(3), sb(4), ALU.mult, ALU.add)
    pvnR = pers.tile([P, U], F32)
    nc.vector.reciprocal(pvnR[:], zR[:])
    lpvnL = pers.tile([P, U], F32)
    nc.scalar.activation(lpvnL[:], pvnL[:], AF.Ln)
    lpvnR = pers.tile([P, U], F32)
    nc.scalar.activation(lpvnR[:], pvnR[:], AF.Ln)
    kc2 = pers.tile([P, U], F32)
    nc.vector.tensor_add(kc2[:], lpvnL[:], lpvnR[:])

    nRf = pers.tile([P, U], F32)
    nc.vector.tensor_scalar(nRf[:], gt[:], -1.0, 8191.0, ALU.mult, ALU.add)
    gc = pers.tile([P, U], F32)
    nc.vector.tensor_scalar_max(gc[:], gt[:], 1.0)
    inv_n1 = pers.tile([P, U], F32)
    nc.vector.reciprocal(inv_n1[:], gc[:])
    nR1c = pers.tile([P, U], F32)
    nc.vector.tensor_scalar(nR1c[:], gt[:], -1.0, 8190.0, ALU.mult, ALU.add)
    nc.vector.tensor_scalar_max(nR1c[:], nR1c[:], 1.0)
    inv_nR1 = pers.tile([P, U], F32)
    nc.vector.reciprocal(inv_nR1[:], nR1c[:])
    inv_n = pers.tile([P, U], F32)
    nc.vector.reciprocal(inv_n[:], nf[:])
    inv_nR = pers.tile([P, U], F32)
    nRc = pers.tile([P, U], F32)
    nc.vector.tensor_scalar_max(nRc[:], nRf[:], 1.0)
    nc.vector.reciprocal(inv_nR[:], nRc[:])

    n_n1 = pers.tile([P, U], F32)
    nc.vector.tensor_mul(n_n1[:], nf[:], inv_n1[:])
    nR_nR1 = pers.tile([P, U], F32)
    nc.vector.tensor_mul(nR_nR1[:], nRf[:], inv_nR1[:])
    i_nn1 = pers.tile([P, U], F32)
    nc.vector.tensor_mul(i_nn1[:], inv_n[:], inv_n1[:])
    i_nRnR1 = pers.tile([P, U], F32)
    nc.vector.tensor_mul(i_nRnR1[:], inv_nR[:], inv_nR1[:])

    CBL = pers.tile([P, U], F32)
    nc.scalar.activation(CBL[:], n_n1[:], AF.Copy, scale=sb(7))
    CBR = pers.tile([P, U], F32)
    nc.scalar.activation(CBR[:], nR_nR1[:], AF.Copy, scale=sb(7))
    CA2L = pers.tile([P, U], F32)
    q1 = pers.tile([P, U], F32)
    nc.scalar.activation(q1[:], pvnL[:], AF.Copy, scale=sb(8))
    q2 = pers.tile([P, U], F32)
    nc.scalar.activation(q2[:], i_nn1[:], AF.Copy, scale=sb(9))
    nc.vector.tensor_add(CA2L[:], q1[:], q2[:])
    CA2R = pers.tile([P, U], F32)
    q1b = pers.tile([P, U], F32)
    nc.scalar.activation(q1b[:], pvnR[:], AF.Copy, scale=sb(8))
    q2b = pers.tile([P, U], F32)
    nc.scalar.activation(q2b[:], i_nRnR1[:], AF.Copy, scale=sb(9))
    nc.vector.tensor_add(CA2R[:], q1b[:], q2b[:])
    CAL = pers.tile([P, U], F32)
    nc.scalar.activation(CAL[:], pvnL[:], AF.Copy, scale=sb(10))
    CAR = pers.tile([P, U], F32)
    nc.scalar.activation(CAR[:], pvnR[:], AF.Copy, scale=sb(10))
    Cc = pers.tile([P, U], F32)
    p12 = pers.tile([P, U], F32)
    nc.vector.tensor_add(p12[:], pvnL[:], pvnR[:])
    cc1 = pers.tile([P, U], F32)
    nc.scalar.activation(cc1[:], p12[:], AF.Copy, scale=sb(11))
    cct = pers.tile([P, U], F32)
    nc.vector.tensor_scalar(cct[:], kc2[:], 0.5, sb(12), ALU.mult, ALU.add)
    nc.vector.tensor_add(Cc[:], cc1[:], cct[:])
    mlo = pers.tile([P, U], F32)
    nc.vector.tensor_scalar(mlo[:], gt[:], 14.5, NEG, ALU.is_lt, ALU.mult)
    mhi = pers.tile([P, U], F32)
    nc.vector.tensor_scalar(mhi[:], gt[:], 8174.5, NEG, ALU.is_ge, ALU.mult)
    nc.vector.tensor_add(Cc[:], Cc[:], mlo[:])
    nc.vector.tensor_add(Cc[:], Cc[:], mhi[:])
    nemask = pers.tile([P, U], F32)
    nc.vector.tensor_scalar(nemask[:], gt[:], 6552.5, None, ALU.is_ge)

    bund = pers.tile([P, 80], F32)
    zeros = pers.tile([P, max(CHUNK_SIZES) * U], F32)
    nc.gpsimd.memset(zeros[:], 0.0)

    Tall = ps1.tile([BL, 5, P], F32)
    Tm = Tall[:, 0, :]
    Tz = Tall[:, 1, :]
    Te = Tall[:, 2, :]
    Ta = Tall[:, 3, :]
    Tb = Tall[:, 4, :]
    M16 = pers.tile([BL, 1], F32)
    d = pers.tile([BL, P], F32)
    w = pers.tile([BL, P], F32)
    bfW = pers.tile([BL, 1], F32)
    sig = pers.tile([BL, 1], F32)

    def trace_bfw():
        nc.tensor.transpose(Ta, bund[:, 48 : 48 + BL], id_t[:])
        nc.tensor.transpose(Tb, bund[:, 64 : 64 + BL], id_t[:])
        At16 = Ta[:, 0:1]
        Bt16 = Tb[:, 0:1]
        t1 = pers.tile([BL, 1], F32)
        nc.scalar.activation(t1[:], At16, AF.Square, scale=1.0 / 32.0)
        v2 = pers.tile([BL, 1], F32)
        nc.vector.tensor_scalar_mul(v2[:], t1[:], 1.0 / 8192.0)
        vW = pers.tile([BL, 1], F32)
        nc.vector.scalar_tensor_tensor(
            vW[:], Bt16, 1.0 / 1024.0, v2[:], ALU.mult, ALU.subtract
        )
        nc.vector.tensor_scalar(vW[:], vW[:], 1.0 / 8191.0, 1.0e-8, ALU.mult, ALU.max)
        term1 = pers.tile([BL, 1], F32)
        nc.vector.tensor_scalar_mul(term1[:], vW[:], sb(17, BL))
        term2 = pers.tile([BL, 1], F32)
        nc.vector.tensor_scalar_mul(term2[:], t1[:], sb(18, BL))
        uu = pers.tile([BL, 1], F32)
        nc.scalar.activation(uu[:], At16, AF.Identity, bias=sb(6, BL), scale=sb(5, BL))
        u2 = pers.tile([BL, 1], F32)
        nc.scalar.activation(u2[:], uu[:], AF.Square)
        term3 = pers.tile([BL, 1], F32)
        nc.vector.tensor_scalar_mul(term3[:], u2[:], sb(13, BL))
        tsum = pers.tile([BL, 1], F32)
        nc.vector.tensor_add(tsum[:], term1[:], term2[:])
        nc.vector.tensor_sub(tsum[:], tsum[:], term3[:])
        nc.vector.tensor_scalar(bfW[:], tsum[:], -0.5, sb(19, BL), ALU.mult, ALU.add)

    def trace_maxw():
        nc.tensor.transpose(Tm, bund[:, 0:BL], id_t[:])
        nc.vector.tensor_reduce(M16[:], Tm, AX.X, ALU.min, negate=True)
        nc.vector.tensor_scalar(d[:], Tm, -1.0, M16[:], ALU.mult, ALU.subtract)
        nc.scalar.activation(w[:], d[:], AF.Exp)
        sigin = pers.tile([BL, 1], F32)
        nc.vector.tensor_sub(sigin[:], M16[:], bfW[:])
        nc.scalar.activation(sig[:], sigin[:], AF.Sigmoid)

    xhs = [None] * len(chunks)

    def ufof(ci):
        return UF_SCHED[ci] if UF_SCHED is not None else 32

    def trace_fold(ci):
        uf = ufof(ci)
        bc = chunks[ci][1]
        if uf > 0:
            xh = wk.tile([P, bc, uf, 16], F32, tag="xh")
            nc.gpsimd.tensor_add(
                xh[:], xts[ci][:, :, 0:uf, 0:16], xts[ci][:, :, 0:uf, 16:32]
            )
            xhs[ci] = xh

    trace_fold(0)
    for ci, (bs, bc) in enumerate(chunks):
        xt = xts[ci]
        last = ci == len(chunks) - 1
        if not last:
            trace_fold(ci + 1)

        uf = ufof(ci)
        sr = wk.tile([P, bc, U], F32)
        if uf > 0:
            if ci == 0 and uf == 16:
                for q in range(1, 4):
                    nc.vector.tensor_reduce(
                        sr[:, :, q * 16 : (q + 1) * 16],
                        xt[:, :, q * 16 : (q + 1) * 16, :],
                        AX.X,
                        ALU.add,
                    )
            elif uf < U:
                nc.vector.tensor_reduce(
                    sr[:, :, uf:U], xt[:, :, uf:U, :], AX.X, ALU.add
                )
            nc.vector.tensor_reduce(sr[:, :, 0:uf], xhs[ci][:], AX.X, ALU.add)
        else:
            HU = U // 2
            nc.vector.tensor_reduce(
                sr[:, :, 0:HU], xt[:, :, 0:HU, :], AX.X, ALU.add
            )
            nc.vector.tensor_reduce(
                sr[:, :, HU:U], xt[:, :, HU:U, :], AX.X, ALU.add
            )
        sq = wk.tile([P, bc, U], F32)
        nc.scalar.activation(sq[:], sr[:], AF.Square)

        A = wk.tile([P, bc, U], F32)
        nc.vector.tensor_tensor_scan(
            A[:].rearrange("p b u -> p (b u)"),
            sr[:].rearrange("p b u -> p (b u)"),
            zeros[:, 0 : bc * U],
            0.0,
            ALU.add,
            ALU.add,
        )
        Bt_ = wk.tile([P, bc, U], F32)
        nc.vector.tensor_tensor_scan(
